# revision 9
# baseline (speedup 1.0000x reference)
"""GATv2 convolution (nn_GATv2Convolution) on 8 Trainium2 NeuronCores.

Strategy
- 8 cores own contiguous 12544-row target-node ranges (no collectives).
- Per core, edges are sorted by (target panel of 128 nodes, source chunk of
  25088 nodes) on the host; per-edge value rows are fetched from on-device
  bf16 tables with dma_gather (int16 local indices, 4 SWDGE queues).
- One-hot matrices built with tensor_scalar(is_equal) against iota tiles
  bridge edges<->nodes on the TensorEngine: q-broadcast (oh_NE @ Qpanel),
  v-add (identity accumulate), and scatter-add (oh_EN^T @ weighted values)
  into a per-panel PSUM accumulator.
- Softmax runs without the segment-max shift: logits are O(6) here, exp is
  safe in fp32/bf16, and the result is mathematically identical.
- leaky_relu(x) = 0.6*(x + (2/3)|x|); the 0.6 is folded into the attention
  weights (for the edge features) and into the normalization reciprocal
  (for the final activation) -- exact, since leaky_relu is positively
  homogeneous.
"""
import sys

sys.path.insert(0, '/opt/trn_rl_repo')

import numpy as np
import ml_dtypes

N_NODES = 100000
D_FEAT = 64
HC = 64
NUM_HEADS = 4
HEAD_CH = 16
ALPHA = 0.2
NCORES = 8
P = 128
DUMMY_REL = 200


class Cfg:
    def __init__(self, n_nodes, range_sz, chunk_sz, smax=8):
        assert range_sz % P == 0 and chunk_sz % P == 0
        self.n_nodes = n_nodes
        self.range = range_sz
        self.n_pad = range_sz * NCORES
        self.panels = range_sz // P
        self.chunk = chunk_sz
        self.n_chunks = (self.n_pad + chunk_sz - 1) // chunk_sz
        self.smax = smax


FULL = Cfg(N_NODES, 12544, 25088)


# --------------------------------------------------------------------- host
def host_prep(cfg, edge_source, edge_target):
    src = np.asarray(edge_source, dtype=np.int64)
    tgt = np.asarray(edge_target, dtype=np.int64)
    core = tgt // cfg.range
    NCH = cfg.n_chunks

    per_counts = np.zeros((NCORES, cfg.panels, NCH), dtype=np.int64)
    core_data = []
    for k in range(NCORES):
        m = core == k
        s_k = src[m]
        t_k = tgt[m] - k * cfg.range
        panel = t_k // P
        rel = t_k % P
        ch = s_k // cfg.chunk
        lidx = s_k % cfg.chunk
        order = np.lexsort((lidx, ch, panel))
        panel, rel, ch, lidx = panel[order], rel[order], ch[order], lidx[order]
        np.add.at(per_counts[k], (panel, ch), 1)
        core_data.append((panel, rel, ch, lidx))

    counts = per_counts.max(axis=0)
    subt = -(-counts // P)
    empty = subt.sum(axis=1) == 0
    subt[empty, 0] = 1

    runs = []            # (panel, chunk, n_subtiles, bucket_subtile_offset)
    for p in range(cfg.panels):
        for c in range(NCH):
            s = int(subt[p, c])
            done = 0
            while s > 0:
                take = min(s, cfg.smax)
                runs.append((p, c, take, done))
                done += take
                s -= take
    T = int(subt.sum())

    gcols, off = [], 0
    for (_, _, s, _) in runs:
        gcols.append(off)
        off += s * 8
    idx_cols = off

    panel_sub = subt.sum(axis=1)
    maxrow = int(panel_sub.max()) * P

    # run -> (rel_w column, rel_f row offset), shared across cores
    run_sub0 = []
    t_cursor = 0
    prow = np.zeros(cfg.panels, dtype=np.int64)
    for (p, c, s, _) in runs:
        run_sub0.append((t_cursor, int(prow[p])))
        prow[p] += s * P
        t_cursor += s

    sched = dict(runs=runs, subt=subt, T=T, gcols=gcols, idx_cols=idx_cols,
                 maxrow=maxrow, panel_sub=panel_sub, run_sub0=run_sub0)

    percore = []
    for k in range(NCORES):
        panel, rel, ch, lidx = core_data[k]
        key = panel * NCH + ch
        starts = np.searchsorted(key, np.arange(cfg.panels * NCH))
        ends = np.searchsorted(key, np.arange(cfg.panels * NCH) + 1)

        idx16 = np.zeros((128, idx_cols), dtype=np.int16)
        rel_w = np.full((128, T), DUMMY_REL, dtype=np.float32)
        rel_f = np.full((128, maxrow), DUMMY_REL, dtype=ml_dtypes.bfloat16)
        for ri, (p, c, s, done) in enumerate(runs):
            a, b = int(starts[p * NCH + c]), int(ends[p * NCH + c])
            lo = a + done * P
            hi = min(b, lo + s * P)
            n = max(0, hi - lo)
            li = np.zeros(s * P, dtype=np.int16)
            re = np.full(s * P, DUMMY_REL, dtype=np.float32)
            if n > 0:
                li[:n] = lidx[lo:hi]
                re[:n] = rel[lo:hi]
            cols = s * 8
            wr = li.reshape(cols, 16).T
            idx16[:, gcols[ri]:gcols[ri] + cols] = np.tile(wr, (8, 1))
            t0, r0 = run_sub0[ri]
            rel_w[:, t0:t0 + s] = re.reshape(s, P).T
            rel_f[p, r0:r0 + s * P] = re.astype(ml_dtypes.bfloat16)
        percore.append(dict(idx16=idx16, rel_w=rel_w, rel_f=rel_f))
    return sched, percore


def build_inputs(cfg, sched, percore, node_features, Wq, bq, Wv, bv, attn_kernel):
    nf = np.zeros((cfg.n_pad, D_FEAT), dtype=np.float32)
    nf[:cfg.n_nodes] = node_features
    nfT = np.concatenate([nf.T, np.ones((1, cfg.n_pad), np.float32)], axis=0)
    nfT = np.ascontiguousarray(nfT).astype(ml_dtypes.bfloat16)
    WqA = np.concatenate([Wq, bq[None, :]], 0).astype(ml_dtypes.bfloat16)
    WvA = np.concatenate([Wv, bv[None, :]], 0).astype(ml_dtypes.bfloat16)
    # attention vector laid out heads-major (col h*16+c = attn[c,h]),
    # pre-scaled by 0.6 (leaky_relu slope fold), tiled smax times.
    a64 = (attn_kernel.T.reshape(-1) * 0.6).astype(np.float32)
    attn_t = np.tile(a64[None, :], (128, cfg.smax)).astype(ml_dtypes.bfloat16)
    ident = np.eye(128, dtype=ml_dtypes.bfloat16)
    in_maps = []
    for k in range(NCORES):
        d = percore[k]
        in_maps.append({
            'nfT': nfT,
            'nfTc': np.ascontiguousarray(nfT[:, k * cfg.range:(k + 1) * cfg.range]),
            'WqA': WqA, 'WvA': WvA, 'attn_t': attn_t, 'ident': ident,
            'idx16': d['idx16'], 'rel_w': d['rel_w'], 'rel_f': d['rel_f'],
        })
    return in_maps


# ------------------------------------------------------------------ program
def build_program(cfg, sched, num_devices=NCORES, dbg=False):
    from concourse import bass, mybir, bacc, library_config
    import concourse.tile as tile

    dt = mybir.dt
    Alu = mybir.AluOpType
    Act = mybir.ActivationFunctionType
    runs = sched['runs']
    T = sched['T']
    gcols = sched['gcols']
    run_sub0 = sched['run_sub0']
    NCH = cfg.n_chunks
    CHROWS = [min(cfg.chunk, cfg.n_pad - c * cfg.chunk) for c in range(NCH)]

    nc = bacc.Bacc("TRN2", target_bir_lowering=False, debug=False,
                   num_devices=num_devices, num_swdge_queues=4)
    t_nfT = nc.dram_tensor('nfT', [D_FEAT + 1, cfg.n_pad], dt.bfloat16,
                           kind='ExternalInput')
    t_nfTc = nc.dram_tensor('nfTc', [D_FEAT + 1, cfg.range], dt.bfloat16,
                            kind='ExternalInput')
    t_WqA = nc.dram_tensor('WqA', [D_FEAT + 1, HC], dt.bfloat16,
                           kind='ExternalInput')
    t_WvA = nc.dram_tensor('WvA', [D_FEAT + 1, HC], dt.bfloat16,
                           kind='ExternalInput')
    t_attn = nc.dram_tensor('attn_t', [128, cfg.smax * HC], dt.bfloat16,
                            kind='ExternalInput')
    t_ident = nc.dram_tensor('ident', [128, 128], dt.bfloat16,
                             kind='ExternalInput')
    t_idx = nc.dram_tensor('idx16', [128, sched['idx_cols']], dt.int16,
                           kind='ExternalInput')
    t_relw = nc.dram_tensor('rel_w', [128, T], dt.float32, kind='ExternalInput')
    t_relf = nc.dram_tensor('rel_f', [128, sched['maxrow']], dt.bfloat16,
                            kind='ExternalInput')
    t_out = nc.dram_tensor('out', [cfg.range, HC], dt.float32,
                           kind='ExternalOutput')
    t_V = [nc.dram_tensor(f'V{c}', [CHROWS[c], 128], dt.bfloat16)
           for c in range(NCH)]
    if dbg:
        t_dbg = {n: nc.dram_tensor(f'dbg_{n}', [128, sz], dt.float32,
                                   kind='ExternalOutput')
                 for n, sz in [('ohne', 1024), ('ohen', 1024), ('sps', 512),
                               ('feat', 512), ('logits', 32), ('expv', 32),
                               ('wv', 8 * 68), ('g', 1024), ('pooled', 68)]}
    t_Q = nc.dram_tensor('Q', [cfg.range, HC], dt.bfloat16)

    with tile.TileContext(nc) as tc:
        nc.gpsimd.load_library(library_config.mlp)
        with (
            tc.tile_pool(name='const', bufs=1) as cpool,
            tc.tile_pool(name='pa', bufs=4) as papool,
            tc.tile_pool(name='pap', bufs=2, space='PSUM') as papsum,
            tc.tile_pool(name='ring', bufs=4) as ring,
            tc.tile_pool(name='ohp', bufs=4) as ohpool,
            tc.tile_pool(name='chain', bufs=3) as chain,
            tc.tile_pool(name='spsum', bufs=2, space='PSUM') as spsum,
            tc.tile_pool(name='plp', bufs=3, space='PSUM') as plpool,
            tc.tile_pool(name='np', bufs=3) as npool,
        ):
            # ---------------- resident tiles / constants
            sb_attn = cpool.tile([128, cfg.smax * HC], dt.bfloat16)
            nc.sync.dma_start(sb_attn[:], t_attn[:])
            sb_id = cpool.tile([128, 128], dt.bfloat16)
            nc.sync.dma_start(sb_id[:], t_ident[:])
            sb_idx = cpool.tile([128, sched['idx_cols']], dt.int16)
            nc.sync.dma_start(sb_idx[:], t_idx[:])
            sb_relw = cpool.tile([128, T], dt.float32)
            nc.sync.dma_start(sb_relw[:], t_relw[:])
            iota_row = cpool.tile([128, 128], dt.bfloat16)
            nc.gpsimd.iota(iota_row[:], pattern=[[1, 128]], base=0,
                           channel_multiplier=0,
                           allow_small_or_imprecise_dtypes=True)
            iota_col = cpool.tile([128, 1], dt.float32)
            nc.gpsimd.iota(iota_col[:], pattern=[[0, 1]], base=0,
                           channel_multiplier=1,
                           allow_small_or_imprecise_dtypes=True)
            sb_wq = cpool.tile([D_FEAT + 1, HC], dt.bfloat16)
            nc.sync.dma_start(sb_wq[:], t_WqA[:])
            sb_wv = cpool.tile([D_FEAT + 1, HC], dt.bfloat16)
            nc.sync.dma_start(sb_wv[:], t_WvA[:])

            # ---------------- phase A: dense node transforms -> DRAM tables
            def dense_table(dst, dst_cols, lhsT_dram, rows, w_tile):
                n_tiles = rows // P
                for g0 in range(0, n_tiles, 8):
                    gn = min(8, n_tiles - g0)
                    ps = papsum.tile([128, 8 * HC], dt.float32, tag='paps')
                    st = papool.tile([128, 8 * HC], dt.bfloat16, tag='past')
                    for i in range(gn):
                        lt = papool.tile([D_FEAT + 1, P], dt.bfloat16, tag='palhs')
                        nc.sync.dma_start(
                            lt[:], lhsT_dram[:, (g0 + i) * P:(g0 + i + 1) * P])
                        nc.tensor.matmul(out=ps[:, i * HC:(i + 1) * HC],
                                         lhsT=lt[:], rhs=w_tile[:],
                                         start=True, stop=True)
                    nc.vector.tensor_copy(st[:, :gn * HC], ps[:, :gn * HC])
                    sview = st[:, :gn * HC].rearrange('p (g c) -> p g c', c=HC)
                    dview = dst[g0 * P:(g0 + gn) * P, 0:HC].rearrange(
                        '(g p) c -> p g c', p=P)
                    nc.sync.dma_start(dview, sview)
                    if dst_cols > HC:
                        dview2 = dst[g0 * P:(g0 + gn) * P, HC:2 * HC].rearrange(
                            '(g p) c -> p g c', p=P)
                        nc.sync.dma_start(dview2, sview)

            dense_table(t_Q, HC, t_nfTc, cfg.range, sb_wq)
            for c in range(NCH):
                dense_table(t_V[c], 128, t_nfT[:, c * cfg.chunk:
                                               c * cfg.chunk + CHROWS[c]],
                            CHROWS[c], sb_wv)

            # ---------------- edge phase
            cur_panel = -1
            qp = None
            pooled = None
            runs_left_in_panel = {}
            for (p, c, s, d) in runs:
                runs_left_in_panel[p] = runs_left_in_panel.get(p, 0) + 1

            for ri, (p, c, s, _) in enumerate(runs):
                t0, r0 = run_sub0[ri]
                if p != cur_panel:
                    cur_panel = p
                    qp = npool.tile([128, HC], dt.bfloat16, tag='qp')
                    nc.sync.dma_start(qp[:], t_Q[p * P:(p + 1) * P, :])
                    pooled = plpool.tile([128, HC + NUM_HEADS], dt.float32,
                                         tag='pooled')
                    first_of_panel = True
                else:
                    first_of_panel = False

                # gathered value rows for this run's s*128 edges
                g = ring.tile([128, cfg.smax, 128], dt.bfloat16, tag='vring')
                nc.gpsimd.dma_gather(
                    out_ap=g[:, :s, :], in_ap=t_V[c][:],
                    idxs_ap=sb_idx[:, gcols[ri]:gcols[ri] + s * 8],
                    num_idxs=s * P, num_idxs_reg=s * P, elem_size=128,
                    queue_num=ri % 4)

                # rel broadcast row [n, e] from DRAM (replicated read)
                rr = ohpool.tile([128, cfg.smax * P], dt.bfloat16, tag='rr')
                nc.scalar.dma_start(
                    rr[:, :s * P],
                    t_relf[p:p + 1, r0:r0 + s * P].to_broadcast([128, s * P]))
                oh_ne = ohpool.tile([128, cfg.smax * P], dt.bfloat16, tag='ohne')
                nc.vector.tensor_scalar(oh_ne[:, :s * P], rr[:, :s * P],
                                        iota_col[:], None, op0=Alu.is_equal)

                s_ps = spsum.tile([128, cfg.smax * HC], dt.float32, tag='sps')
                oh_en = ohpool.tile([128, cfg.smax * P], dt.bfloat16, tag='ohen')
                for k in range(s):
                    nc.vector.tensor_scalar(
                        oh_en[:, k * P:(k + 1) * P], iota_row[:],
                        sb_relw[:, t0 + k:t0 + k + 1], None, op0=Alu.is_equal)
                for k in range(s):
                    nc.tensor.matmul(out=s_ps[:, k * HC:(k + 1) * HC],
                                     lhsT=oh_ne[:, k * P:(k + 1) * P],
                                     rhs=qp[:], start=(k == 0), stop=False,
                                     skip_group_check=True)
                for k in range(s):
                    nc.tensor.matmul(out=s_ps[:, k * HC:(k + 1) * HC],
                                     lhsT=sb_id[:], rhs=g[:, k, 0:HC],
                                     start=False, stop=(k == s - 1),
                                     skip_group_check=True)

                # feat' = s + (2/3)|s|   (leaky_relu / 0.6)
                ab = chain.tile([128, cfg.smax * HC], dt.bfloat16, tag='ab')
                nc.scalar.activation(ab[:, :s * HC], s_ps[:, :s * HC],
                                     Act.Abs, scale=2.0 / 3.0)
                feat = chain.tile([128, cfg.smax * HC], dt.bfloat16, tag='feat')
                nc.vector.scalar_tensor_tensor(
                    feat[:, :s * HC], ab[:, :s * HC], 1.0, s_ps[:, :s * HC],
                    op0=Alu.mult, op1=Alu.add)
                prod = chain.tile([128, cfg.smax * HC], dt.bfloat16, tag='prod')
                nc.vector.tensor_tensor(out=prod[:, :s * HC],
                                        in0=feat[:, :s * HC],
                                        in1=sb_attn[:, :s * HC], op=Alu.mult)
                logits = chain.tile([128, cfg.smax * NUM_HEADS], dt.float32,
                                    tag='lg')
                nc.vector.tensor_reduce(
                    logits[:, :s * NUM_HEADS],
                    prod[:, :s * HC].rearrange('p (a b) -> p a b', b=HEAD_CH),
                    axis=mybir.AxisListType.X, op=Alu.add)
                expv = chain.tile([128, cfg.smax * NUM_HEADS], dt.bfloat16,
                                  tag='expv')
                nc.scalar.activation(expv[:, :s * NUM_HEADS],
                                     logits[:, :s * NUM_HEADS], Act.Exp)
                expx = chain.tile([128, cfg.smax * NUM_HEADS, HEAD_CH],
                                  dt.bfloat16, tag='expx')
                nc.gpsimd.tensor_copy(
                    expx[:, :s * NUM_HEADS, :],
                    expv[:, :s * NUM_HEADS].rearrange(
                        'p (a one) -> p a one', one=1).to_broadcast(
                        [128, s * NUM_HEADS, HEAD_CH]))
                wv = chain.tile([128, cfg.smax, HC + NUM_HEADS], dt.bfloat16,
                                tag='wv')
                nc.vector.tensor_tensor(
                    out=wv[:, :s, 0:HC],
                    in0=g[:, :s, 0:HC],
                    in1=expx[:, :s * NUM_HEADS, :].rearrange(
                        'p (a b) c -> p a (b c)', b=NUM_HEADS),
                    op=Alu.mult)
                nc.vector.tensor_copy(
                    wv[:, :s, HC:],
                    expv[:, :s * NUM_HEADS].rearrange(
                        'p (a b) -> p a b', b=NUM_HEADS))

                for k in range(s):
                    nc.tensor.matmul(out=pooled[:],
                                     lhsT=oh_en[:, k * P:(k + 1) * P],
                                     rhs=wv[:, k, :],
                                     start=(first_of_panel and k == 0),
                                     stop=(runs_left_in_panel[p] == 1 and
                                           k == s - 1),
                                     skip_group_check=True)
                if dbg and ri == 0:
                    def _tap(name, ap):
                        st = chain.tile([128, t_dbg[name].shape[1]],
                                        dt.float32, tag=f'dbg{name}')
                        n = min(ap.free_size(), st.free_size())
                        a2 = ap.rearrange(' '.join(
                            ['p'] + [chr(97 + i) for i in range(len(ap.shape) - 1)])
                            + ' -> p (' + ' '.join(
                            [chr(97 + i) for i in range(len(ap.shape) - 1)]) + ')') \
                            if len(ap.shape) > 2 else ap
                        nc.vector.tensor_copy(st[:, :n], a2[:, :n] if True else a2)
                        nc.sync.dma_start(t_dbg[name][:, :n], st[:, :n])
                    _tap('ohne', oh_ne[:, :s * P])
                    _tap('ohen', oh_en[:, :s * P])
                    _tap('sps', s_ps[:, :s * HC])
                    _tap('feat', feat[:, :s * HC])
                    _tap('logits', logits[:, :s * NUM_HEADS])
                    _tap('expv', expv[:, :s * NUM_HEADS])
                    _tap('wv', wv[:, :s, :])
                    _tap('g', g[:, :s, :])
                runs_left_in_panel[p] -= 1

                if runs_left_in_panel[p] == 0:
                    dn = npool.tile([128, NUM_HEADS], dt.float32, tag='dn')
                    nc.vector.tensor_scalar_max(dn[:], pooled[:, HC:], 1e-12)
                    rc = npool.tile([128, NUM_HEADS], dt.float32, tag='rc')
                    nc.vector.reciprocal(rc[:], dn[:])
                    nc.vector.tensor_scalar_mul(rc[:], rc[:], 0.6)
                    on = npool.tile([128, HC], dt.float32, tag='on')
                    nc.vector.tensor_tensor(
                        out=on[:].rearrange('p (h c) -> p h c', c=HEAD_CH),
                        in0=pooled[:, 0:HC].rearrange('p (h c) -> p h c',
                                                      c=HEAD_CH),
                        in1=rc[:].rearrange('p (h one) -> p h one',
                                            one=1).to_broadcast(
                            [128, NUM_HEADS, HEAD_CH]),
                        op=Alu.mult)
                    ab2 = npool.tile([128, HC], dt.float32, tag='ab2')
                    nc.scalar.activation(ab2[:], on[:], Act.Abs,
                                         scale=2.0 / 3.0)
                    of = npool.tile([128, HC], dt.float32, tag='of')
                    nc.vector.tensor_tensor(out=of[:], in0=on[:], in1=ab2[:],
                                            op=Alu.add)
                    nc.sync.dma_start(t_out[p * P:(p + 1) * P, :], of[:])
                    if dbg and p == 0:
                        stp = npool.tile([128, HC + NUM_HEADS], dt.float32,
                                         tag='dbgpl')
                        nc.vector.tensor_copy(stp[:], pooled[:])
                        nc.sync.dma_start(t_dbg['pooled'][:, :HC + NUM_HEADS],
                                          stp[:])

    nc.compile()
    return nc


# -------------------------------------------------------------------- entry
def kernel(node_features, Wq, bq, Wv, bv, attn_kernel, edge_source,
           edge_target, _cfg=None, _run=None):
    cfg = _cfg or FULL
    node_features = np.asarray(node_features, dtype=np.float32)
    Wq = np.asarray(Wq, dtype=np.float32)
    bq = np.asarray(bq, dtype=np.float32)
    Wv = np.asarray(Wv, dtype=np.float32)
    bv = np.asarray(bv, dtype=np.float32)
    attn_kernel = np.asarray(attn_kernel, dtype=np.float32)
    edge_source = np.asarray(edge_source, dtype=np.int64)
    edge_target = np.asarray(edge_target, dtype=np.int64)

    sched, percore = host_prep(cfg, edge_source, edge_target)
    in_maps = build_inputs(cfg, sched, percore, node_features, Wq, bq, Wv, bv,
                           attn_kernel)
    nc = build_program(cfg, sched)

    if _run is None:
        from concourse.bass_utils import run_bass_kernel_spmd
        res = run_bass_kernel_spmd(nc, in_maps, list(range(NCORES)))
        outs = [res.results[k]['out'] for k in range(NCORES)]
    else:
        outs, _ = _run(nc, in_maps)
    full = np.concatenate(outs, axis=0)[:cfg.n_nodes]
    return full.astype(np.float32)


# revision 10
# speedup vs baseline: 1.1369x; 1.1369x over previous
"""GATv2 convolution (nn_GATv2Convolution) on 8 Trainium2 NeuronCores.

Strategy
- 8 cores own contiguous 12544-row target-node ranges (no collectives).
- Per core, edges are sorted by (target panel of 128 nodes, source chunk of
  25088 nodes) on the host; per-edge value rows are fetched from on-device
  bf16 tables with dma_gather (int16 local indices, 4 SWDGE queues).
- One-hot matrices built with tensor_scalar(is_equal) against iota tiles
  bridge edges<->nodes on the TensorEngine: q-broadcast (oh_NE @ Qpanel),
  v-add (identity accumulate), and scatter-add (oh_EN^T @ weighted values)
  into a per-panel PSUM accumulator.
- Softmax runs without the segment-max shift: logits are O(6) here, exp is
  safe in fp32/bf16, and the result is mathematically identical.
- leaky_relu(x) = 0.6*(x + (2/3)|x|); the 0.6 is folded into the attention
  weights (for the edge features) and into the normalization reciprocal
  (for the final activation) -- exact, since leaky_relu is positively
  homogeneous.
"""
import sys

sys.path.insert(0, '/opt/trn_rl_repo')

import numpy as np
import ml_dtypes

N_NODES = 100000
D_FEAT = 64
HC = 64
NUM_HEADS = 4
HEAD_CH = 16
ALPHA = 0.2
NCORES = 8
P = 128
DUMMY_REL = -1


class Cfg:
    def __init__(self, n_nodes, range_sz, chunk_sz, smax=8):
        assert range_sz % P == 0 and chunk_sz % P == 0
        self.n_nodes = n_nodes
        self.range = range_sz
        self.n_pad = range_sz * NCORES
        self.panels = range_sz // P
        self.chunk = chunk_sz
        self.n_chunks = (self.n_pad + chunk_sz - 1) // chunk_sz
        self.smax = smax


FULL = Cfg(N_NODES, 12544, 25088)


# --------------------------------------------------------------------- host
def host_prep(cfg, edge_source, edge_target):
    src = np.asarray(edge_source, dtype=np.int64)
    tgt = np.asarray(edge_target, dtype=np.int64)
    core = tgt // cfg.range
    NCH = cfg.n_chunks

    per_counts = np.zeros((NCORES, cfg.panels, NCH), dtype=np.int64)
    core_data = []
    for k in range(NCORES):
        m = core == k
        s_k = src[m]
        t_k = tgt[m] - k * cfg.range
        panel = t_k // P
        rel = t_k % P
        ch = s_k // cfg.chunk
        lidx = s_k % cfg.chunk
        order = np.lexsort((lidx, ch, panel))
        panel, rel, ch, lidx = panel[order], rel[order], ch[order], lidx[order]
        np.add.at(per_counts[k], (panel, ch), 1)
        core_data.append((panel, rel, ch, lidx))

    counts = per_counts.max(axis=0)
    subt = -(-counts // P)
    empty = subt.sum(axis=1) == 0
    subt[empty, 0] = 1

    runs = []            # (panel, chunk, n_subtiles, bucket_subtile_offset)
    for p in range(cfg.panels):
        for c in range(NCH):
            s = int(subt[p, c])
            done = 0
            while s > 0:
                take = min(s, cfg.smax)
                runs.append((p, c, take, done))
                done += take
                s -= take
    T = int(subt.sum())

    gcols, off = [], 0
    for (_, _, s, _) in runs:
        gcols.append(off)
        off += s * 8
    idx_cols = off

    panel_sub = subt.sum(axis=1)
    maxrow = int(panel_sub.max()) * P

    # run -> (rel_w column, rel_f row offset), shared across cores
    run_sub0 = []
    t_cursor = 0
    prow = np.zeros(cfg.panels, dtype=np.int64)
    for (p, c, s, _) in runs:
        run_sub0.append((t_cursor, int(prow[p])))
        prow[p] += s * P
        t_cursor += s

    sched = dict(runs=runs, subt=subt, T=T, gcols=gcols, idx_cols=idx_cols,
                 maxrow=maxrow, panel_sub=panel_sub, run_sub0=run_sub0)

    percore = []
    for k in range(NCORES):
        panel, rel, ch, lidx = core_data[k]
        key = panel * NCH + ch
        starts = np.searchsorted(key, np.arange(cfg.panels * NCH))
        ends = np.searchsorted(key, np.arange(cfg.panels * NCH) + 1)

        idx16 = np.zeros((128, idx_cols), dtype=np.int16)
        rel_w = np.full((128, T), DUMMY_REL, dtype=np.float32)
        rel_f = np.full((128, maxrow), DUMMY_REL, dtype=np.int8)
        for ri, (p, c, s, done) in enumerate(runs):
            a, b = int(starts[p * NCH + c]), int(ends[p * NCH + c])
            lo = a + done * P
            hi = min(b, lo + s * P)
            n = max(0, hi - lo)
            li = np.zeros(s * P, dtype=np.int16)
            re = np.full(s * P, DUMMY_REL, dtype=np.float32)
            if n > 0:
                li[:n] = lidx[lo:hi]
                re[:n] = rel[lo:hi]
            cols = s * 8
            wr = li.reshape(cols, 16).T
            idx16[:, gcols[ri]:gcols[ri] + cols] = np.tile(wr, (8, 1))
            t0, r0 = run_sub0[ri]
            rel_w[:, t0:t0 + s] = re.reshape(s, P).T
            rel_f[p, r0:r0 + s * P] = re.astype(np.int8)
        percore.append(dict(idx16=idx16, rel_w=rel_w, rel_f=rel_f))
    return sched, percore


def build_inputs(cfg, sched, percore, node_features, Wq, bq, Wv, bv, attn_kernel):
    nf = np.zeros((cfg.n_pad, D_FEAT), dtype=np.float32)
    nf[:cfg.n_nodes] = node_features
    nfT = np.concatenate([nf.T, np.ones((1, cfg.n_pad), np.float32)], axis=0)
    nfT = np.ascontiguousarray(nfT).astype(ml_dtypes.bfloat16)
    WqA = np.concatenate([Wq, bq[None, :]], 0).astype(ml_dtypes.bfloat16)
    WvA = np.concatenate([Wv, bv[None, :]], 0).astype(ml_dtypes.bfloat16)
    # attention vector laid out heads-major (col h*16+c = attn[c,h]),
    # pre-scaled by 0.6 (leaky_relu slope fold), tiled smax times.
    a64 = (attn_kernel.T.reshape(-1) * 0.6).astype(np.float32)
    attn_t = np.tile(a64[None, :], (128, cfg.smax)).astype(ml_dtypes.bfloat16)
    ident = np.eye(128, dtype=ml_dtypes.bfloat16)
    in_maps = []
    for k in range(NCORES):
        d = percore[k]
        in_maps.append({
            'nfT': nfT,
            'nfTc': np.ascontiguousarray(nfT[:, k * cfg.range:(k + 1) * cfg.range]),
            'WqA': WqA, 'WvA': WvA, 'attn_t': attn_t, 'ident': ident,
            'idx16': d['idx16'], 'rel_w': d['rel_w'], 'rel_f': d['rel_f'],
        })
    return in_maps


# ------------------------------------------------------------------ program
def build_program(cfg, sched, num_devices=NCORES, dbg=False):
    from concourse import bass, mybir, bacc, library_config
    import concourse.tile as tile

    dt = mybir.dt
    Alu = mybir.AluOpType
    Act = mybir.ActivationFunctionType
    runs = sched['runs']
    T = sched['T']
    gcols = sched['gcols']
    run_sub0 = sched['run_sub0']
    NCH = cfg.n_chunks
    CHROWS = [min(cfg.chunk, cfg.n_pad - c * cfg.chunk) for c in range(NCH)]

    nc = bacc.Bacc("TRN2", target_bir_lowering=False, debug=False,
                   num_devices=num_devices, num_swdge_queues=4)
    t_nfT = nc.dram_tensor('nfT', [D_FEAT + 1, cfg.n_pad], dt.bfloat16,
                           kind='ExternalInput')
    t_nfTc = nc.dram_tensor('nfTc', [D_FEAT + 1, cfg.range], dt.bfloat16,
                            kind='ExternalInput')
    t_WqA = nc.dram_tensor('WqA', [D_FEAT + 1, HC], dt.bfloat16,
                           kind='ExternalInput')
    t_WvA = nc.dram_tensor('WvA', [D_FEAT + 1, HC], dt.bfloat16,
                           kind='ExternalInput')
    t_attn = nc.dram_tensor('attn_t', [128, cfg.smax * HC], dt.bfloat16,
                            kind='ExternalInput')
    t_ident = nc.dram_tensor('ident', [128, 128], dt.bfloat16,
                             kind='ExternalInput')
    t_idx = nc.dram_tensor('idx16', [128, sched['idx_cols']], dt.int16,
                           kind='ExternalInput')
    t_relw = nc.dram_tensor('rel_w', [128, T], dt.float32, kind='ExternalInput')
    t_relf = nc.dram_tensor('rel_f', [128, sched['maxrow']], dt.int8,
                            kind='ExternalInput')
    t_out = nc.dram_tensor('out', [cfg.range, HC], dt.float32,
                           kind='ExternalOutput')
    t_V = [nc.dram_tensor(f'V{c}', [CHROWS[c], 128], dt.bfloat16)
           for c in range(NCH)]
    if dbg:
        t_dbg = {n: nc.dram_tensor(f'dbg_{n}', [128, sz], dt.float32,
                                   kind='ExternalOutput')
                 for n, sz in [('ohne', 1024), ('ohen', 1024), ('sps', 512),
                               ('feat', 512), ('logits', 32), ('expv', 32),
                               ('wv', 8 * 68), ('g', 1024), ('pooled', 68)]}
    t_Q = nc.dram_tensor('Q', [cfg.range, HC], dt.bfloat16)

    with tile.TileContext(nc) as tc:
        nc.gpsimd.load_library(library_config.mlp)
        with (
            tc.tile_pool(name='const', bufs=1) as cpool,
            tc.tile_pool(name='pa', bufs=4) as papool,
            tc.tile_pool(name='pap', bufs=2, space='PSUM') as papsum,
            tc.tile_pool(name='ring', bufs=4) as ring,
            tc.tile_pool(name='ohp', bufs=4) as ohpool,
            tc.tile_pool(name='chain', bufs=3) as chain,
            tc.tile_pool(name='spsum', bufs=2, space='PSUM') as spsum,
            tc.tile_pool(name='plp', bufs=3, space='PSUM') as plpool,
            tc.tile_pool(name='np', bufs=3) as npool,
        ):
            # ---------------- resident tiles / constants
            sb_attn = cpool.tile([128, cfg.smax * HC], dt.bfloat16)
            nc.sync.dma_start(sb_attn[:], t_attn[:])
            sb_id = cpool.tile([128, 128], dt.bfloat16)
            nc.sync.dma_start(sb_id[:], t_ident[:])
            sb_idx = cpool.tile([128, sched['idx_cols']], dt.int16)
            nc.sync.dma_start(sb_idx[:], t_idx[:])
            sb_relw = cpool.tile([128, T], dt.float32)
            nc.sync.dma_start(sb_relw[:], t_relw[:])
            iota_row = cpool.tile([128, 128], dt.bfloat16)
            nc.gpsimd.iota(iota_row[:], pattern=[[1, 128]], base=0,
                           channel_multiplier=0,
                           allow_small_or_imprecise_dtypes=True)
            iota_col = cpool.tile([128, 1], dt.float32)
            nc.gpsimd.iota(iota_col[:], pattern=[[0, 1]], base=0,
                           channel_multiplier=1,
                           allow_small_or_imprecise_dtypes=True)
            sb_wq = cpool.tile([D_FEAT + 1, HC], dt.bfloat16)
            nc.sync.dma_start(sb_wq[:], t_WqA[:])
            sb_wv = cpool.tile([D_FEAT + 1, HC], dt.bfloat16)
            nc.sync.dma_start(sb_wv[:], t_WvA[:])

            # ---------------- phase A: dense node transforms -> DRAM tables
            def dense_table(dst, dst_cols, lhsT_dram, rows, w_tile):
                n_tiles = rows // P
                for g0 in range(0, n_tiles, 8):
                    gn = min(8, n_tiles - g0)
                    ps = papsum.tile([128, 8 * HC], dt.float32, tag='paps')
                    st = papool.tile([128, 8 * HC], dt.bfloat16, tag='past')
                    for i in range(gn):
                        lt = papool.tile([D_FEAT + 1, P], dt.bfloat16, tag='palhs')
                        nc.sync.dma_start(
                            lt[:], lhsT_dram[:, (g0 + i) * P:(g0 + i + 1) * P])
                        nc.tensor.matmul(out=ps[:, i * HC:(i + 1) * HC],
                                         lhsT=lt[:], rhs=w_tile[:],
                                         start=True, stop=True)
                    nc.vector.tensor_copy(st[:, :gn * HC], ps[:, :gn * HC])
                    sview = st[:, :gn * HC].rearrange('p (g c) -> p g c', c=HC)
                    dview = dst[g0 * P:(g0 + gn) * P, 0:HC].rearrange(
                        '(g p) c -> p g c', p=P)
                    nc.sync.dma_start(dview, sview)
                    if dst_cols > HC:
                        dview2 = dst[g0 * P:(g0 + gn) * P, HC:2 * HC].rearrange(
                            '(g p) c -> p g c', p=P)
                        nc.sync.dma_start(dview2, sview)

            dense_table(t_Q, HC, t_nfTc, cfg.range, sb_wq)
            for c in range(NCH):
                dense_table(t_V[c], 128, t_nfT[:, c * cfg.chunk:
                                               c * cfg.chunk + CHROWS[c]],
                            CHROWS[c], sb_wv)

            # ---------------- edge phase
            cur_panel = -1
            qp = None
            pooled = None
            runs_left_in_panel = {}
            for (p, c, s, d) in runs:
                runs_left_in_panel[p] = runs_left_in_panel.get(p, 0) + 1

            for ri, (p, c, s, _) in enumerate(runs):
                t0, r0 = run_sub0[ri]
                if p != cur_panel:
                    cur_panel = p
                    qp = npool.tile([128, HC], dt.bfloat16, tag='qp')
                    nc.sync.dma_start(qp[:], t_Q[p * P:(p + 1) * P, :])
                    pooled = plpool.tile([128, HC + NUM_HEADS], dt.float32,
                                         tag='pooled')
                    first_of_panel = True
                else:
                    first_of_panel = False

                # gathered value rows for this run's s*128 edges
                g = ring.tile([128, cfg.smax, 128], dt.bfloat16, tag='vring')
                nc.gpsimd.dma_gather(
                    out_ap=g[:, :s, :], in_ap=t_V[c][:],
                    idxs_ap=sb_idx[:, gcols[ri]:gcols[ri] + s * 8],
                    num_idxs=s * P, num_idxs_reg=s * P, elem_size=128,
                    queue_num=ri % 4)

                # rel broadcast row [n, e] from DRAM (replicated read)
                rr = ohpool.tile([128, cfg.smax * P], dt.int8, tag='rr')
                nc.scalar.dma_start(
                    rr[:, :s * P],
                    t_relf[p:p + 1, r0:r0 + s * P].to_broadcast([128, s * P]))
                oh_ne = ohpool.tile([128, cfg.smax * P], dt.bfloat16, tag='ohne')
                nc.vector.tensor_scalar(oh_ne[:, :s * P], rr[:, :s * P],
                                        iota_col[:], None, op0=Alu.is_equal)

                s_ps = spsum.tile([128, cfg.smax * HC], dt.float32, tag='sps')
                oh_en = ohpool.tile([128, cfg.smax * P], dt.bfloat16, tag='ohen')
                for k in range(s):
                    nc.vector.tensor_scalar(
                        oh_en[:, k * P:(k + 1) * P], iota_row[:],
                        sb_relw[:, t0 + k:t0 + k + 1], None, op0=Alu.is_equal)
                for k in range(s):
                    nc.tensor.matmul(out=s_ps[:, k * HC:(k + 1) * HC],
                                     lhsT=oh_ne[:, k * P:(k + 1) * P],
                                     rhs=qp[:], start=(k == 0), stop=False,
                                     skip_group_check=True)
                for k in range(s):
                    nc.tensor.matmul(out=s_ps[:, k * HC:(k + 1) * HC],
                                     lhsT=sb_id[:], rhs=g[:, k, 0:HC],
                                     start=False, stop=(k == s - 1),
                                     skip_group_check=True)

                # feat' = s + (2/3)|s|   (leaky_relu / 0.6)
                ab = chain.tile([128, cfg.smax * HC], dt.bfloat16, tag='ab')
                nc.scalar.activation(ab[:, :s * HC], s_ps[:, :s * HC],
                                     Act.Abs, scale=2.0 / 3.0)
                feat = chain.tile([128, cfg.smax * HC], dt.bfloat16, tag='feat')
                nc.vector.scalar_tensor_tensor(
                    feat[:, :s * HC], ab[:, :s * HC], 1.0, s_ps[:, :s * HC],
                    op0=Alu.mult, op1=Alu.add)
                prod = chain.tile([128, cfg.smax * HC], dt.bfloat16, tag='prod')
                nc.vector.tensor_tensor(out=prod[:, :s * HC],
                                        in0=feat[:, :s * HC],
                                        in1=sb_attn[:, :s * HC], op=Alu.mult)
                logits = chain.tile([128, cfg.smax * NUM_HEADS], dt.float32,
                                    tag='lg')
                nc.vector.tensor_reduce(
                    logits[:, :s * NUM_HEADS],
                    prod[:, :s * HC].rearrange('p (a b) -> p a b', b=HEAD_CH),
                    axis=mybir.AxisListType.X, op=Alu.add)
                expv = chain.tile([128, cfg.smax * NUM_HEADS], dt.bfloat16,
                                  tag='expv')
                nc.scalar.activation(expv[:, :s * NUM_HEADS],
                                     logits[:, :s * NUM_HEADS], Act.Exp)
                wv = chain.tile([128, cfg.smax, HC + NUM_HEADS], dt.bfloat16,
                                tag='wv')
                nc.vector.tensor_tensor(
                    out=wv[:, :s, 0:HC].rearrange('p a (h c) -> p a h c',
                                                  c=HEAD_CH),
                    in0=g[:, :s, 0:HC].rearrange('p a (h c) -> p a h c',
                                                 c=HEAD_CH),
                    in1=expv[:, :s * NUM_HEADS].rearrange(
                        'p (a h one) -> p a h one', h=NUM_HEADS,
                        one=1).to_broadcast(
                        [128, s, NUM_HEADS, HEAD_CH]),
                    op=Alu.mult)
                nc.vector.tensor_copy(
                    wv[:, :s, HC:],
                    expv[:, :s * NUM_HEADS].rearrange(
                        'p (a b) -> p a b', b=NUM_HEADS))

                for k in range(s):
                    nc.tensor.matmul(out=pooled[:],
                                     lhsT=oh_en[:, k * P:(k + 1) * P],
                                     rhs=wv[:, k, :],
                                     start=(first_of_panel and k == 0),
                                     stop=(runs_left_in_panel[p] == 1 and
                                           k == s - 1),
                                     skip_group_check=True)
                if dbg and ri == 0:
                    def _tap(name, ap):
                        st = chain.tile([128, t_dbg[name].shape[1]],
                                        dt.float32, tag=f'dbg{name}')
                        n = min(ap.free_size(), st.free_size())
                        a2 = ap.rearrange(' '.join(
                            ['p'] + [chr(97 + i) for i in range(len(ap.shape) - 1)])
                            + ' -> p (' + ' '.join(
                            [chr(97 + i) for i in range(len(ap.shape) - 1)]) + ')') \
                            if len(ap.shape) > 2 else ap
                        nc.vector.tensor_copy(st[:, :n], a2[:, :n] if True else a2)
                        nc.sync.dma_start(t_dbg[name][:, :n], st[:, :n])
                    _tap('ohne', oh_ne[:, :s * P])
                    _tap('ohen', oh_en[:, :s * P])
                    _tap('sps', s_ps[:, :s * HC])
                    _tap('feat', feat[:, :s * HC])
                    _tap('logits', logits[:, :s * NUM_HEADS])
                    _tap('expv', expv[:, :s * NUM_HEADS])
                    _tap('wv', wv[:, :s, :])
                    _tap('g', g[:, :s, :])
                runs_left_in_panel[p] -= 1

                if runs_left_in_panel[p] == 0:
                    dn = npool.tile([128, NUM_HEADS], dt.float32, tag='dn')
                    nc.vector.tensor_scalar_max(dn[:], pooled[:, HC:], 1e-12)
                    rc = npool.tile([128, NUM_HEADS], dt.float32, tag='rc')
                    nc.vector.reciprocal(rc[:], dn[:])
                    nc.vector.tensor_scalar_mul(rc[:], rc[:], 0.6)
                    on = npool.tile([128, HC], dt.float32, tag='on')
                    nc.vector.tensor_tensor(
                        out=on[:].rearrange('p (h c) -> p h c', c=HEAD_CH),
                        in0=pooled[:, 0:HC].rearrange('p (h c) -> p h c',
                                                      c=HEAD_CH),
                        in1=rc[:].rearrange('p (h one) -> p h one',
                                            one=1).to_broadcast(
                            [128, NUM_HEADS, HEAD_CH]),
                        op=Alu.mult)
                    ab2 = npool.tile([128, HC], dt.float32, tag='ab2')
                    nc.scalar.activation(ab2[:], on[:], Act.Abs,
                                         scale=2.0 / 3.0)
                    of = npool.tile([128, HC], dt.float32, tag='of')
                    nc.vector.tensor_tensor(out=of[:], in0=on[:], in1=ab2[:],
                                            op=Alu.add)
                    nc.sync.dma_start(t_out[p * P:(p + 1) * P, :], of[:])
                    if dbg and p == 0:
                        stp = npool.tile([128, HC + NUM_HEADS], dt.float32,
                                         tag='dbgpl')
                        nc.vector.tensor_copy(stp[:], pooled[:])
                        nc.sync.dma_start(t_dbg['pooled'][:, :HC + NUM_HEADS],
                                          stp[:])

    nc.compile()
    return nc


# -------------------------------------------------------------------- entry
def kernel(node_features, Wq, bq, Wv, bv, attn_kernel, edge_source,
           edge_target, _cfg=None, _run=None):
    cfg = _cfg or FULL
    node_features = np.asarray(node_features, dtype=np.float32)
    Wq = np.asarray(Wq, dtype=np.float32)
    bq = np.asarray(bq, dtype=np.float32)
    Wv = np.asarray(Wv, dtype=np.float32)
    bv = np.asarray(bv, dtype=np.float32)
    attn_kernel = np.asarray(attn_kernel, dtype=np.float32)
    edge_source = np.asarray(edge_source, dtype=np.int64)
    edge_target = np.asarray(edge_target, dtype=np.int64)

    sched, percore = host_prep(cfg, edge_source, edge_target)
    in_maps = build_inputs(cfg, sched, percore, node_features, Wq, bq, Wv, bv,
                           attn_kernel)
    nc = build_program(cfg, sched)

    if _run is None:
        from concourse.bass_utils import run_bass_kernel_spmd
        res = run_bass_kernel_spmd(nc, in_maps, list(range(NCORES)))
        outs = [res.results[k]['out'] for k in range(NCORES)]
    else:
        outs, _ = _run(nc, in_maps)
    full = np.concatenate(outs, axis=0)[:cfg.n_nodes]
    return full.astype(np.float32)
